# revision 11
# baseline (speedup 1.0000x reference)
"""NomicBertAttention on 8 Trainium2 NeuronCores.

Sharding: 8-way head tensor-parallelism (2 heads/core, both batches),
per-batch AllToAll to re-shard ctx by sequence rows, then row-parallel
out-proj + residual + LayerNorm (each core owns 512 of 4096 rows).

Key implementation choices (v2):
- all projection / scores / out-proj matmuls in bf16 (inputs quantized
  host-side); attention context matmul in fp8e4m3 with DoubleRow perf
  mode (2 t-chunks of 128 contracted per pass at 2 cols/cycle).
- RoPE rotate-half done with a single stream_shuffle: features of each
  head are permuted host-side to pair-adjacent order [f0,f32,f1,f33,..]
  so the rotation is an in-quadrant pair swap (mask[i]=i^1). cos/sin
  tables are permuted + sign-folded to match. scores are invariant to
  a shared q/k feature permutation.
- softmax denominator via a ones-column in the fp8 v tile (row 65 of
  ctx PSUM); reciprocal via the fast custom-DVE approx (~18 bits).
- phase B software-pipelined: ctx matmul of t-pair p emitted after the
  scores of pair p+1 so the PE never waits on the ACT exp latency.
- AllToAll split per batch (dummy zero slots for the other batch's
  windows); the first A2A overlaps batch-1 attention. Phase C combines
  both A2A outputs with one add (zeros where not owned).
- k bias dropped (softmax-invariant); v bias and out bias folded into
  the residual host-side; q bias kept via a ones-row matmul.
"""

import numpy as np
import concourse.bacc as bacc
import concourse.mybir as mybir
import concourse.tile as tile
from concourse.bass_utils import run_bass_kernel_spmd
from concourse.masks import make_identity

F32 = mybir.dt.float32
BF16 = mybir.dt.bfloat16
F8E4 = mybir.dt.float8e4
MULT = mybir.AluOpType.mult
ADD = mybir.AluOpType.add
SUB = mybir.AluOpType.subtract
BYPASS = mybir.AluOpType.bypass
EXP = mybir.ActivationFunctionType.Exp
SQRT = mybir.ActivationFunctionType.Sqrt
IDENT = mybir.ActivationFunctionType.Identity
DR = mybir.MatmulPerfMode.DoubleRow

B, S, D, H, HD = 2, 2048, 1024, 16, 64
NC = 8
HPC = H // NC          # 2 heads per core
F = HPC * HD           # 128 projected features per core
ROWS = B * S // NC     # 512 output rows per core
NSEQ = B * S           # 4096 flattened rows
NT = NSEQ // 128       # 32 global t-chunks
NPAIR = NT // 2        # 16 global t-pairs
PB = S // 256          # 8 t-pairs per batch
EPS = 1e-12

LAST_RESULTS = None

SWAP_MASK = [i ^ 1 for i in range(32)]


def _build():
    nc = bacc.Bacc("TRN2", target_bir_lowering=False, debug=False, num_devices=NC)

    xaT = nc.dram_tensor("xaT", [D + 1, NSEQ], BF16, kind="ExternalInput")
    wq = nc.dram_tensor("wq", [D, F], BF16, kind="ExternalInput")
    wk = nc.dram_tensor("wk", [D, F], BF16, kind="ExternalInput")
    wv = nc.dram_tensor("wv", [D, F], BF16, kind="ExternalInput")
    wqb = nc.dram_tensor("wqb", [1, F], BF16, kind="ExternalInput")
    woT = nc.dram_tensor("woT", [D, D], BF16, kind="ExternalInput")
    cs2d = nc.dram_tensor("cs2", [128, S], BF16, kind="ExternalInput")
    sn2d = nc.dram_tensor("sn2", [128, S], BF16, kind="ExternalInput")
    residd = nc.dram_tensor("resid", [ROWS, D], F32, kind="ExternalInput")
    lnwd = nc.dram_tensor("lnw", [128, D], F32, kind="ExternalInput")
    lnbd = nc.dram_tensor("lnb", [128, D], F32, kind="ExternalInput")
    outd = nc.dram_tensor("out", [ROWS, D], F32, kind="ExternalOutput")
    import os as _os
    _dump = bool(_os.environ.get("KD_DUMP"))
    # Defaults are the hardware-validated config: fp8 DoubleRow ctx ON;
    # fp32 transposes, reciprocal (exact), no custom-DVE/TTR, shuffle fed
    # from SBUF. The KD_* flags opt back into the risky variants.
    _ex_bf16 = bool(_os.environ.get("KD_EX_BF16"))
    _no_dr = _ex_bf16 or bool(_os.environ.get("KD_NO_DR"))
    _shuf_sbuf = not _os.environ.get("KD_SHUF_PSUM")
    _slow_recip = not _os.environ.get("KD_FAST_RECIP")
    _trp_f32 = not _os.environ.get("KD_TRP_BF16")
    _no_ttr = not _os.environ.get("KD_TTR")
    EXDT = BF16 if _ex_bf16 else F8E4
    TRDT = F32 if _trp_f32 else BF16
    if _dump:
        qSo = nc.dram_tensor("qSo", [128, NSEQ], BF16, kind="ExternalOutput")
        kSo = nc.dram_tensor("kSo", [128, NSEQ], BF16, kind="ExternalOutput")
        cfo = nc.dram_tensor("cfo", [128, NSEQ], BF16, kind="ExternalOutput")

    with tile.TileContext(nc) as tc:
        with (
            tc.tile_pool(name="qk", bufs=1) as qkpool,
            tc.tile_pool(name="wpool", bufs=1) as wpool,
            tc.tile_pool(name="xpool", bufs=2) as xpool,
            tc.tile_pool(name="rope", bufs=2) as rpool,
            tc.tile_pool(name="exps", bufs=2) as epool,
            tc.tile_pool(name="ctxp", bufs=2) as ctxpool,
            tc.tile_pool(name="bcastp", bufs=2) as bpool,
            tc.tile_pool(name="small", bufs=4) as spool,
            tc.tile_pool(name="cweights", bufs=1) as cwpool,
            tc.tile_pool(name="psS", bufs=2, space="PSUM") as psS,
            tc.tile_pool(name="psC", bufs=4, space="PSUM") as psC,
            tc.tile_pool(name="dram", bufs=1, space="DRAM") as dpool,
        ):
            # resident RoPE'd projections [feat(2 heads, pair-permuted), pos]
            qS = qkpool.tile([128, NSEQ], BF16, tag="qS")
            kS = qkpool.tile([128, NSEQ], BF16, tag="kS")
            # v in fp8, t-pair DoubleRow layout: [128(t%128), pair, j, 65]
            # col 64 of each (pair, j) is the ones column (softmax denom).
            # (pair, j) block padded to stride 80: DoubleRow ldweights
            # requires the outer free step to be even and 16-aligned.
            vaug = [
                qkpool.tile([128, NPAIR * 160], EXDT, tag=f"vaug{h}", name=f"vaug{h}")
                for h in range(HPC)
            ]
            vviews = []
            for h in range(HPC):
                v4 = vaug[h][:].rearrange("p (pg j m) -> p pg j m", j=2, m=80)
                vviews.append(v4)
                nc.vector.memset(v4[:, :, :, 64:65], 1.0)

            # ---- A2A bounce buffers (split per batch; dummy slots zeroed)
            # payload is bf16 but the collective moves f32-typed bytes
            # (bf16 collectives are not supported on this runtime path)
            a2a_in = [
                dpool.tile([NC, 128, 256], F32, tag=f"a2a_in{b}", name=f"a2a_in{b}")
                for b in range(B)
            ]
            a2a_out = [
                dpool.tile(
                    [NC, 128, 256], F32, tag=f"a2a_out{b}", name=f"a2a_out{b}"
                )
                for b in range(B)
            ]
            zt = wpool.tile([128, 256], F32, tag="zt")
            nc.vector.memset(zt[:], 0.0)
            for j in range(4):
                nc.sync.dma_start(a2a_in[0][4 + j], zt[:])
                nc.sync.dma_start(a2a_in[1][j], zt[:])

            # ---- phase C weights prefetch (wo, ln params)
            won = {}
            for n in range(2):
                for k in range(8):
                    wot = cwpool.tile([128, 512], BF16, tag=f"wo{n}{k}", name=f"wo{n}{k}")
                    nc.sync.dma_start(
                        wot[:], woT[128 * k : 128 * (k + 1), 512 * n : 512 * (n + 1)]
                    )
                    won[n, k] = wot
            lnw = cwpool.tile([128, D], F32, tag="lnw")
            lnb = cwpool.tile([128, D], F32, tag="lnb")
            nc.sync.dma_start(lnw[:], lnwd[:])
            nc.sync.dma_start(lnb[:], lnbd[:])
            eps_t = spool.tile([128, 1], F32, tag="eps_t", bufs=1)
            nc.vector.memset(eps_t[:], EPS)

            # ---- phase A constants
            cs2 = wpool.tile([128, S], BF16, tag="cs2")
            sn2 = wpool.tile([128, S], BF16, tag="sn2")
            nc.sync.dma_start(cs2[:], cs2d[:])
            nc.sync.dma_start(sn2[:], sn2d[:])
            ident = wpool.tile([128, 128], TRDT, tag="ident")
            make_identity(nc, ident[:])
            wsb = {}
            for name, dram_w in (("q", wq), ("k", wk), ("v", wv)):
                wt = wpool.tile([128, D], BF16, tag=f"w{name}", name=f"w{name}")
                for k in range(8):
                    nc.sync.dma_start(
                        wt[:, 128 * k : 128 * (k + 1)],
                        dram_w[128 * k : 128 * (k + 1), :],
                    )
                wsb[name] = wt
            wqbt = wpool.tile([1, F], BF16, tag="wqbt")
            nc.sync.dma_start(wqbt[:], wqb[:])

            # ---- phase A group emitter (one group = 512 positions)
            def emit_group(g):
                gs, ge = 512 * g, 512 * (g + 1)
                cg = (512 * g) % S
                xg = xpool.tile([128, 4096], BF16, tag="xg")
                xone = xpool.tile([1, 512], BF16, tag="xone")
                for k in range(8):
                    nc.sync.dma_start(
                        xg[:, 512 * k : 512 * (k + 1)],
                        xaT[128 * k : 128 * (k + 1), gs:ge],
                    )
                nc.sync.dma_start(xone[:], xaT[D : D + 1, gs:ge])

                for name in ("q", "k", "v"):
                    pp = psS.tile([128, 1024], F32, tag="sc", name="pp")
                    for k in range(8):
                        nc.tensor.matmul(
                            pp[:, 0:512],
                            wsb[name][:, 128 * k : 128 * (k + 1)],
                            xg[:, 512 * k : 512 * (k + 1)],
                            start=(k == 0),
                            stop=(name != "q" and k == 7),
                        )
                    if name == "q":
                        nc.tensor.matmul(
                            pp[:, 0:512], wqbt[:], xone[:], start=False, stop=True
                        )

                    if name in ("q", "k"):
                        dst = qS if name == "q" else kS
                        rot = rpool.tile([128, 512], F32, tag="rot")
                        if _shuf_sbuf:
                            ppc = rpool.tile([128, 512], F32, tag="ppc")
                            nc.vector.tensor_copy(ppc[:], pp[:, 0:512])
                            nc.vector.stream_shuffle(rot[:], ppc[:], SWAP_MASK)
                        else:
                            nc.vector.stream_shuffle(rot[:], pp[:, 0:512], SWAP_MASK)
                        tcos = rpool.tile([128, 512], F32, tag="tcos")
                        nc.vector.tensor_tensor(
                            out=tcos[:], in0=pp[:, 0:512],
                            in1=cs2[:, cg : cg + 512], op=MULT,
                        )
                        tsin = rpool.tile([128, 512], F32, tag="tsin")
                        nc.vector.tensor_tensor(
                            out=tsin[:], in0=rot[:],
                            in1=sn2[:, cg : cg + 512], op=MULT,
                        )
                        nc.vector.tensor_tensor(
                            out=dst[:, gs:ge], in0=tcos[:], in1=tsin[:], op=ADD
                        )
                    else:
                        vTg = rpool.tile([128, 512], TRDT, tag="vTg")
                        nc.vector.tensor_copy(vTg[:], pp[:, 0:512])
                        trp = psS.tile([128, 512], TRDT, tag="sc", name="trp")
                        for sub in range(4):
                            nc.tensor.transpose(
                                trp[:, 128 * sub : 128 * (sub + 1)],
                                vTg[:, 128 * sub : 128 * (sub + 1)],
                                ident[:],
                            )
                        tv = trp[:].rearrange("p (a b f2) -> p a b f2", a=2, b=2)
                        for h in range(HPC):
                            nc.vector.tensor_copy(
                                vviews[h][:, 2 * g : 2 * g + 2, :, 0:64],
                                tv[:, :, :, 64 * h : 64 * h + 64],
                            )

            # ---- phase B window emitter (1024-wide s-window of batch b)
            def emit_window(b, gw, boundary_groups):
                sw = S * b + 1024 * gw
                j0 = 4 * b + 2 * gw
                cps = {}
                for h in range(HPC):
                    for half in range(2):
                        cps[h, half] = psC.tile(
                            [65, 512], F32, tag="cp", name=f"cp_{h}_{half}"
                        )
                prev_ex = None
                for pair in range(PB + 1):
                    if pair < PB:
                        exs = [
                            epool.tile([128, 2048], EXDT, tag=f"ex{h}", name=f"ex{h}")
                            for h in range(HPC)
                        ]
                        for j in range(2):
                            tg = S * b + 256 * pair + 128 * j
                            for h in range(HPC):
                                hs_, he = 64 * h, 64 * (h + 1)
                                sc = psS.tile([128, 1024], F32, tag="sc")
                                for half in range(2):
                                    s0 = sw + 512 * half
                                    nc.tensor.matmul(
                                        sc[:, 512 * half : 512 * (half + 1)],
                                        kS[hs_:he, tg : tg + 128],
                                        qS[hs_:he, s0 : s0 + 512],
                                        start=True,
                                        stop=True,
                                    )
                                nc.scalar.activation(
                                    exs[h][:, 1024 * j : 1024 * (j + 1)], sc[:], EXP
                                )
                    else:
                        exs = None
                    if prev_ex is not None:
                        pex, ppair = prev_ex
                        pg = PB * b + ppair
                        for h in range(HPC):
                            exv = pex[h][:].rearrange("p (j n) -> p j n", j=2)
                            for half in range(2):
                                if _no_dr:
                                    for jj in range(2):
                                        nc.tensor.matmul(
                                            cps[h, half][:],
                                            vviews[h][:, pg, jj, 0:65],
                                            exv[:, jj, 512 * half : 512 * (half + 1)],
                                            start=(ppair == 0 and jj == 0),
                                            stop=(ppair == PB - 1 and jj == 1),
                                        )
                                else:
                                    nc.tensor.matmul(
                                        cps[h, half][:],
                                        vviews[h][:, pg, :, 0:65],
                                        exv[:, :, 512 * half : 512 * (half + 1)],
                                        start=(ppair == 0),
                                        stop=(ppair == PB - 1),
                                        perf_mode=DR,
                                    )
                    prev_ex = (exs, pair) if exs is not None else None
                    if pair < len(boundary_groups):
                        emit_group(boundary_groups[pair])

                # normalize + ship
                for half in range(2):
                    j = j0 + half
                    ctile = ctxpool.tile([128, 512], BF16, tag="ctile")
                    for h in range(HPC):
                        rden = spool.tile([1, 512], F32, tag="rden")
                        if _slow_recip:
                            nc.vector.reciprocal(rden[:], cps[h, half][64:65, :])
                        else:
                            nc.vector.reciprocal_approx_fast(
                                out=rden[:], in_=cps[h, half][64:65, :]
                            )
                        bc = bpool.tile([64, 512], F32, tag="bc")
                        nc.gpsimd.partition_broadcast(bc[:], rden[:])
                        nc.vector.tensor_tensor(
                            out=ctile[64 * h : 64 * (h + 1), :],
                            in0=cps[h, half][0:64, :],
                            in1=bc[:],
                            op=MULT,
                        )
                    nc.sync.dma_start(a2a_in[b][j], ctile[:].bitcast(F32))

            # ---- schedule: A(b0) -> B(b0) [A(b1) interleaved] -> A2A#1
            #                -> B(b1) -> A2A#2 -> C
            for g in range(4):
                emit_group(g)
            _noa2a = bool(_os.environ.get("KD_NOA2A"))

            def do_a2a(b):
                if _noa2a:
                    nc.sync.dma_start(a2a_out[b][:], a2a_in[b][:])
                else:
                    nc.gpsimd.collective_compute(
                        "AllToAll", BYPASS,
                        replica_groups=[list(range(NC))],
                        ins=[a2a_in[b][:]], outs=[a2a_out[b][:]],
                    )

            emit_window(0, 0, [4])
            emit_window(0, 1, [5, 6, 7])
            do_a2a(0)
            emit_window(1, 0, [])
            emit_window(1, 1, [])
            do_a2a(1)

            # ---- phase C: combine, out-proj, residual + LayerNorm
            with tc.tile_pool(name="tail", bufs=2) as lpool:
                ctxA = lpool.tile([128, 4096], BF16, tag="ctxA", bufs=1)
                ctxB = lpool.tile([128, 4096], BF16, tag="ctxB", bufs=1)
                ctxF = lpool.tile([128, 4096], BF16, tag="ctxF", bufs=1)
                for i in range(NC):
                    nc.sync.dma_start(
                        ctxA[:, 512 * i : 512 * (i + 1)].bitcast(F32), a2a_out[0][i]
                    )
                    nc.sync.dma_start(
                        ctxB[:, 512 * i : 512 * (i + 1)].bitcast(F32), a2a_out[1][i]
                    )
                nc.vector.tensor_tensor(out=ctxF[:], in0=ctxA[:], in1=ctxB[:], op=ADD)
                if _dump:
                    nc.sync.dma_start(qSo[:], qS[:])
                    nc.sync.dma_start(kSo[:], kS[:])
                    nc.sync.dma_start(cfo[:], ctxF[:])

                for m in range(4):
                    rsb = lpool.tile([128, D], F32, tag="rsb")
                    nc.sync.dma_start(rsb[:], residd[128 * m : 128 * (m + 1), :])
                    osb = lpool.tile([128, D], F32, tag="osb")
                    accs = [spool.tile([128, 1], F32, tag=f"acc{n}", name=f"acc{n}") for n in range(2)]
                    acc_q = spool.tile([128, 1], F32, tag="acc_q")
                    for n in range(2):
                        op = psS.tile([128, 1024], F32, tag="sc", name="op")
                        for k in range(8):
                            nc.tensor.matmul(
                                op[:, 0:512],
                                ctxF[:, 512 * k + 128 * m : 512 * k + 128 * (m + 1)],
                                won[n, k][:],
                                start=(k == 0),
                                stop=(k == 7),
                            )
                        if _no_ttr:
                            nc.vector.tensor_tensor(
                                out=osb[:, 512 * n : 512 * (n + 1)],
                                in0=op[:, 0:512],
                                in1=rsb[:, 512 * n : 512 * (n + 1)],
                                op=ADD,
                            )
                        else:
                            nc.vector.tensor_tensor_reduce(
                                out=osb[:, 512 * n : 512 * (n + 1)],
                                in0=op[:, 0:512],
                                in1=rsb[:, 512 * n : 512 * (n + 1)],
                                scale=1.0,
                                scalar=0.0,
                                op0=ADD,
                                op1=ADD,
                                accum_out=accs[n][:],
                            )
                    scr = lpool.tile([128, D], F32, tag="scr")
                    acc_s = spool.tile([128, 1], F32, tag="acc_s")
                    if _no_ttr:
                        nc.vector.tensor_reduce(
                            acc_s[:], osb[:], mybir.AxisListType.X, ADD
                        )
                        nc.vector.tensor_tensor(out=scr[:], in0=osb[:], in1=osb[:], op=MULT)
                        nc.vector.tensor_reduce(
                            acc_q[:], scr[:], mybir.AxisListType.X, ADD
                        )
                    else:
                        nc.vector.tensor_tensor_reduce(
                            out=scr[:],
                            in0=osb[:],
                            in1=osb[:],
                            scale=1.0,
                            scalar=0.0,
                            op0=MULT,
                            op1=ADD,
                            accum_out=acc_q[:],
                        )
                        nc.vector.tensor_tensor(
                            out=acc_s[:], in0=accs[0][:], in1=accs[1][:], op=ADD
                        )
                    mean = spool.tile([128, 1], F32, tag="mean")
                    nc.vector.tensor_scalar(mean[:], acc_s[:], 1.0 / D, None, MULT)
                    msq = spool.tile([128, 1], F32, tag="msq")
                    nc.vector.tensor_scalar(msq[:], acc_q[:], 1.0 / D, None, MULT)
                    m2 = spool.tile([128, 1], F32, tag="m2")
                    nc.vector.tensor_tensor(out=m2[:], in0=mean[:], in1=mean[:], op=MULT)
                    var = spool.tile([128, 1], F32, tag="var")
                    nc.vector.tensor_tensor(out=var[:], in0=msq[:], in1=m2[:], op=SUB)
                    sdt = spool.tile([128, 1], F32, tag="sdt")
                    nc.scalar.activation(sdt[:], var[:], SQRT, bias=eps_t[:])
                    rstd = spool.tile([128, 1], F32, tag="rstd")
                    nc.vector.reciprocal(rstd[:], sdt[:])
                    mr = spool.tile([128, 1], F32, tag="mr")
                    nc.vector.tensor_tensor(out=mr[:], in0=mean[:], in1=rstd[:], op=MULT)
                    negmr = spool.tile([128, 1], F32, tag="negmr")
                    nc.vector.tensor_scalar(negmr[:], mr[:], -1.0, None, MULT)
                    onrm = lpool.tile([128, D], F32, tag="onrm")
                    nc.scalar.activation(
                        onrm[:], osb[:], IDENT, bias=negmr[:], scale=rstd[:]
                    )
                    ow = lpool.tile([128, D], F32, tag="ow")
                    nc.vector.tensor_tensor(out=ow[:], in0=onrm[:], in1=lnw[:], op=MULT)
                    ofin = lpool.tile([128, D], F32, tag="ofin")
                    nc.vector.tensor_tensor(out=ofin[:], in0=ow[:], in1=lnb[:], op=ADD)
                    nc.sync.dma_start(outd[128 * m : 128 * (m + 1), :], ofin[:])

    nc.finalize()
    return nc


def _perm_cols():
    """Pair-adjacent feature order within each head: [f0,f32,f1,f33,...]."""
    p = np.empty(HD, dtype=np.int64)
    p[0::2] = np.arange(32)
    p[1::2] = np.arange(32) + 32
    return p


def kernel(hidden_states, cos, sin, Wq, bq, Wk, bk, Wv, bv, Wo, bo, ln_w, ln_b):
    global LAST_RESULTS
    import ml_dtypes

    bf16 = ml_dtypes.bfloat16
    hs = np.ascontiguousarray(np.asarray(hidden_states, np.float32).reshape(NSEQ, D))
    cos = np.asarray(cos, np.float32)
    sin = np.asarray(sin, np.float32)
    Wq = np.asarray(Wq, np.float32)
    bq = np.asarray(bq, np.float32)
    Wk = np.asarray(Wk, np.float32)
    Wv = np.asarray(Wv, np.float32)
    bv = np.asarray(bv, np.float32)
    Wo = np.asarray(Wo, np.float32)
    bo = np.asarray(bo, np.float32)
    ln_w = np.asarray(ln_w, np.float32)
    ln_b = np.asarray(ln_b, np.float32)

    perm = _perm_cols()
    sign = np.where(perm < 32, -1.0, 1.0).astype(np.float32)[:, None]  # [64,1]
    # cos/sin rows permuted to the pair-adjacent feature order, sign folded
    cosT = cos.T[perm]                           # [64, S]
    sinT = sin.T[perm] * sign                    # [64, S]
    cs2 = np.ascontiguousarray(np.concatenate([cosT, cosT], axis=0)).astype(bf16)
    sn2 = np.ascontiguousarray(np.concatenate([sinT, sinT], axis=0)).astype(bf16)

    xaT = np.ascontiguousarray(
        np.concatenate([hs.T, np.ones((1, NSEQ), np.float32)], axis=0)
    ).astype(bf16)
    lnw_t = np.ascontiguousarray(np.tile(ln_w[None, :], (128, 1)))
    lnb_t = np.ascontiguousarray(np.tile(ln_b[None, :], (128, 1)))
    woT = np.ascontiguousarray(Wo.T).astype(bf16)            # [din, dout]
    wq_s = (Wq / 64.0).T                                     # fold SCALING^2
    bq_s = bq / 64.0
    wkT = Wk.T
    wvT = Wv.T
    resid_base = hs + bo[None, :] + (bv @ Wo.T)[None, :]     # fold bv, bo

    # per-head column permutation applied to q/k weight slices
    def permute_heads(w2d):  # [*, F] -> pair-adjacent order per 64-block
        out = np.empty_like(w2d)
        for h in range(HPC):
            blk = w2d[..., 64 * h : 64 * (h + 1)]
            out[..., 64 * h : 64 * (h + 1)] = blk[..., perm]
        return out

    in_maps = []
    for c in range(NC):
        fs = slice(F * c, F * (c + 1))
        rs = slice(ROWS * c, ROWS * (c + 1))
        in_maps.append(
            {
                "xaT": xaT,
                "wq": np.ascontiguousarray(permute_heads(wq_s[:, fs])).astype(bf16),
                "wk": np.ascontiguousarray(permute_heads(wkT[:, fs])).astype(bf16),
                "wv": np.ascontiguousarray(wvT[:, fs]).astype(bf16),
                "wqb": np.ascontiguousarray(permute_heads(bq_s[None, fs])).astype(bf16),
                "woT": woT,
                "cs2": cs2,
                "sn2": sn2,
                "resid": np.ascontiguousarray(resid_base[rs]),
                "lnw": lnw_t,
                "lnb": lnb_t,
            }
        )

    nc = _build()
    LAST_RESULTS = run_bass_kernel_spmd(nc, in_maps, core_ids=list(range(NC)))
    out = np.concatenate([LAST_RESULTS.results[c]["out"] for c in range(NC)], axis=0)
    return out.reshape(B, S, D)
